# revision 13
# baseline (speedup 1.0000x reference)
"""Fused single-head attention + residual + LayerNorm for Trainium2 (Bass/Tile).

Problem: B=4, S=4096, E=512 fp32.
  Q/K/V = x @ W^T + b ; S = QK^T/sqrt(E) ; mask keys ; softmax ; ctx = P@V ;
  out = LayerNorm(ctx + x) * gamma + beta

Sharding: 8 cores = 4 batches x 2 halves of the S=4096 query rows
(sequence-parallel attention). Masked keys carry exactly zero softmax
weight, so each core receives only the PACKED (unmasked) key rows of its
whole batch.

Kernel strategy (v4):
  - The Q and K projections are FUSED on the host: scores^T = x_k G x_q^T
    with G = Wq^T Wk precomputed in fp32 and quantized once (x64 scaling
    keeps G out of the fp8 subnormal range; the 1/64 rides the exp scale).
    The device computes T = G x_q^T (same cost as the old Q projection)
    and contracts it against the resident x_k fp8 tiles -- the whole K
    projection disappears. bq enters scores only as a per-key bias
    bq.K[k] = x_k.(Wk^T bq), folded into the host-built mask-bias vector
    in exact fp32; bk is per-query (softmax-invariant, dropped); bv is
    pre-added to the bf16 residual. Wv is scaled x32 (fp8 subnormal
    range); the P-rowsum ones-column is 32 so one reciprocal cancels all.
  - Every input arrives as a per-tile SBUF image (partition-major,
    contiguous) so each load is a dense 128-descriptor DMA; operand tiles
    are chunked to the order the PE consumes them, issues are spread over
    the sync/scalar/gpsimd queues, and the 2MB bf16 residual is deferred.
    Gap-free feeding also keeps the PE out of its cold p-state (a stall
    resets a ~3us ramp to full clock).
  - Middle phase: transposed score tiles [128k, 2qc, 512q] cycle through a
    3-deep 6-bank PSUM ring, paced by the ScalarE exp wall (~1us per
    half-tile, table pre-warmed at t=0); the V projection runs in its own
    1-deep 2-bank ring, emitted after each 2-k-tile group's score matmuls
    so it fills PE slack without delaying exp. P is written straight into
    the paired fp8 ctx layout.
  - ctx accumulates over all k-tiles per 128-query chunk; rowsum rides as
    V column 512. LayerNorm is spread over four engines so no queue runs
    above the PE's 1.9us/chunk pace: DVE recip/h-half/split bn_stats,
    ScalarE scale-half + sqrt + normalize-half, GpSimd residual-add +
    -mu*rstd + normalize-half, output DMA issued from the idle sync queue.
    bn_stats runs per half so stats begin before the residual-add of the
    second half lands.
"""

import sys

import ml_dtypes
import numpy as np

sys.path.insert(0, "/opt/trn_rl_repo")

import concourse.bass as bass  # noqa: E402
import concourse.tile as tile  # noqa: E402
from concourse import bacc, mybir  # noqa: E402

E = 512
S = 4096  # keys per batch
SQ = 2048  # query rows per core
QC = SQ // 512  # 4   512-chunks along q
F32 = mybir.dt.float32
BF16 = mybir.dt.bfloat16
FP8 = mybir.dt.float8e4
SCALE = 1.0 / float(np.sqrt(E))
G_SCALE = 64.0  # G quantized as 64*(Wq^T Wk); exp scale divides it out
V_SCALE = 32.0  # Wv quantized as 32*Wv^T; ones-col=32 cancels via recip
EPS = 1e-5
MASK_NEG = -10000.0
SHIFT = -1.0  # softmax-invariant score shift, keeps exp() in fp8 range
DR = mybir.MatmulPerfMode.DoubleRow


def build_nc(nkt_real, nkt2, apply_gb):
    # nkt_real = k-tiles of 128 packed keys; nkt2 = even-rounded (DR pairs)
    assert nkt2 % 2 == 0 and nkt_real in (nkt2, nkt2 - 1)
    SK = nkt2 * 128
    JP = nkt2 // 2  # ctx pair-tiles of 256 keys
    KC = (nkt2 + 3) // 4  # xkv 512-column chunks

    nc = bacc.Bacc("TRN2", target_bir_lowering=False, debug=False)
    # inputs are per-tile SBUF images: [128 partitions, tile-free...] dense
    xr = nc.dram_tensor("xr", [128, 16 * E], BF16, kind="ExternalInput")
    xq8d = nc.dram_tensor("xq8", [128, 2 * QC * 2 * 512], FP8, kind="ExternalInput")
    xkv8d = nc.dram_tensor(
        "xkv8", [128, 2 * KC * 2 * 512], FP8, kind="ExternalInput"
    )
    mbd = nc.dram_tensor("mb", [128, nkt2], F32, kind="ExternalInput")
    G8d = nc.dram_tensor("G8", [128, 2048], FP8, kind="ExternalInput")
    Wv8d = nc.dram_tensor("Wv8", [128, 2048], FP8, kind="ExternalInput")
    gamma = nc.dram_tensor("gamma", [E], F32, kind="ExternalInput")
    beta = nc.dram_tensor("beta", [E], F32, kind="ExternalInput")
    out = nc.dram_tensor("out", [SQ, E], BF16, kind="ExternalOutput")

    AF = mybir.ActivationFunctionType
    OP = mybir.AluOpType

    with tile.TileContext(nc) as tc:
        with tc.tile_pool(name="persist", bufs=1) as persist:
            # ---------------- constants / persistent tiles ----------------
            mbcols = persist.tile([128, nkt2], F32, tag="mb")
            eps_t = persist.tile([128, 1], F32, tag="eps")
            nc.vector.memset(eps_t, EPS)
            dummy = persist.tile([128, 1], F32, tag="dummy")
            if apply_gb:
                ga_bc = persist.tile([128, E], F32, tag="gabc")
                be_bc = persist.tile([128, E], F32, tag="bebc")
                nc.gpsimd.dma_start(
                    out=ga_bc, in_=bass.AP(tensor=gamma[:].tensor, offset=0, ap=[[0, 128], [1, E]])
                )
                nc.gpsimd.dma_start(
                    out=be_bc, in_=bass.AP(tensor=beta[:].tensor, offset=0, ap=[[0, 128], [1, E]])
                )

            # fp8 operand tiles (paired [.., 2, ..] layouts)
            # g8/wv8: [128e, 2(mc), 2(i), 512f]; contract e = 256mc + 128i + p
            g8 = persist.tile([128, 2, 2, E], FP8, name="g8", tag="g8")
            wv8 = persist.tile([128, 2, 2, E], FP8, name="wv8", tag="wv8")
            # xq8[mc][qc]: [128e, 2(i), 512q]
            xq8 = [
                [
                    persist.tile(
                        [128, 2, 512], FP8, name=f"xq8_{m}_{c}", tag=f"xq8_{m}_{c}"
                    )
                    for c in range(QC)
                ]
                for m in range(2)
            ]
            # xkv8[mc][c]: [128e, 2(i), 512k]  (k-chunk of 4 k-tiles)
            xkv8 = [
                [
                    persist.tile(
                        [128, 2, 512], FP8, name=f"xkv8_{m}_{c}", tag=f"xkv8_{m}_{c}"
                    )
                    for c in range(KC)
                ]
                for m in range(2)
            ]
            # T8[mo]: [128f, 2(i), SQ]  (T = 64*G x_q^T; f = 256mo + 128i + p)
            T8 = [
                persist.tile([128, 2, SQ], FP8, name=f"T8_{m}", tag=f"T8_{m}")
                for m in range(2)
            ]
            # v8: [128k, JP, 2(i), 513]  (k = 256jp + 128i + p; col 512 = 32)
            v8 = persist.tile([128, JP, 2, E + 1], FP8, name="v8", tag="v8")
            # P tiles: pt[jp]: [128k, 4(qc), 2(i), 512q]
            pt = [
                persist.tile([128, 4, 2, 512], FP8, name=f"pt{j}", tag=f"pt{j}")
                for j in range(JP)
            ]
            # residual x staged for the LN phase: [128, 16(qi), 512]
            xqst = persist.tile([128, 16, E], BF16, name="xqst", tag="xqst")

            # ---- critical loads first: weights lead the scalar queue,
            # xq8 per-tile chunks on sync, xkv8 on gpsimd, residual deferred
            nc.sync.dma_start(out=g8, in_=G8d[:, :])
            nc.scalar.dma_start(out=wv8, in_=Wv8d[:, :])
            for qc in range(QC):
                for m in range(2):
                    nc.sync.dma_start(
                        out=xq8[m][qc],
                        in_=xq8d[:, (m * QC + qc) * 1024 : (m * QC + qc + 1) * 1024],
                    )
            for c in range(KC):
                for m in range(2):
                    nc.gpsimd.dma_start(
                        out=xkv8[m][c],
                        in_=xkv8d[:, (m * KC + c) * 1024 : (m * KC + c + 1) * 1024],
                    )
            nc.gpsimd.dma_start(out=mbcols, in_=mbd[:, :])

            # pre-warm the exp activation table while DMAs stream
            nc.scalar.activation(out=dummy, in_=eps_t, func=AF.Exp)

            # rowsum ones-columns (=32 to cancel the Wv x32) on idle GpSimd
            nc.gpsimd.memset(v8[:, :, :, E : E + 1], V_SCALE)
            if nkt_real < nkt2:
                # pad k-tile never exp'd: zero its P so ctx sees 0 weight
                nc.gpsimd.memset(pt[JP - 1][:, :, 1, :], 0.0)
            # deferred bulk: bf16 residual (first needed by the LN phase)
            nc.gpsimd.dma_start(out=xqst, in_=xr[:, :])

            def kslice(kt):
                c, r = kt // 4, kt % 4
                return [xkv8[m][c][:, :, r * 128 : (r + 1) * 128] for m in range(2)]

            with (
                tc.tile_pool(name="sp", bufs=3, space="PSUM") as spp,
                tc.tile_pool(name="vp", bufs=1, space="PSUM") as vpp,
            ):
                def t_tile(qc, mo):
                    ps = spp.tile([128, 2, 512], F32, tag="sc")
                    for i in range(2):
                        ft = 2 * mo + i
                        for mc in range(2):
                            nc.tensor.matmul(
                                ps[:, i, :],
                                g8[:, mc, :, ft * 128 : (ft + 1) * 128],
                                xq8[mc][qc],
                                start=(mc == 0),
                                stop=(mc == 1),
                                perf_mode=DR,
                            )
                    dst = T8[mo][:, :, qc * 512 : (qc + 1) * 512]
                    # first half (qc 0,1) on ScalarE so its queue is clear
                    # before the exps begin; second half on DVE in parallel
                    if qc < 2:
                        nc.scalar.copy(out=dst, in_=ps)
                    else:
                        nc.vector.tensor_copy(dst, ps)

                def score_half(kt, h):
                    ks = kslice(kt)
                    sc = spp.tile([128, 2, 512], F32, tag="sc")
                    for mc in range(2):
                        for qq in range(2):
                            qc = 2 * h + qq
                            nc.tensor.matmul(
                                sc[:, qq, :],
                                ks[mc],
                                T8[mc][:, :, qc * 512 : (qc + 1) * 512],
                                start=(mc == 0),
                                stop=(mc == 1),
                                perf_mode=DR,
                            )
                    nc.scalar.activation(
                        out=pt[kt // 2][:, 2 * h : 2 * h + 2, kt % 2, :],
                        in_=sc,
                        func=AF.Exp,
                        bias=mbcols[:, kt : kt + 1],
                        scale=SCALE / G_SCALE,
                    )

                def v_group(g):
                    # V projection for k-tiles 2g, 2g+1 (PE stall-gap filler,
                    # own 1-deep ring so it never blocks a score tile)
                    ps = vpp.tile([128, 2, 512], F32, tag="vt")
                    for i in range(2):
                        if 2 * g + i >= nkt2:
                            continue
                        ks = kslice(2 * g + i)
                        for mc in range(2):
                            nc.tensor.matmul(
                                ps[:, i, :],
                                ks[mc],
                                wv8[:, mc, :, :],
                                start=(mc == 0),
                                stop=(mc == 1),
                                perf_mode=DR,
                            )
                    nc.vector.tensor_copy(v8[:, g, :, 0:E], ps)

                # ---- T = G x_q^T (fused QK projection), with the first
                # score half squeezed in after T tiles 0-3 so the ScalarE
                # exp wall starts as early as possible ----
                for qc in range(2):
                    for mo in range(2):
                        t_tile(qc, mo)
                score_half(0, 0)
                for qc in range(2, QC):
                    for mo in range(2):
                        t_tile(qc, mo)

                # ---- scores + exp (ScalarE wall) with V-proj interleaved ----
                score_half(0, 1)
                for g in range(JP):
                    for i in range(2):
                        kt = 2 * g + i
                        if kt >= nkt_real:
                            continue
                        for h in range(2):
                            if kt == 0:
                                continue  # emitted above
                            score_half(kt, h)
                    v_group(g)

            # ---------------- ctx + residual + layernorm ----------------
            with (
                tc.tile_pool(name="cx", bufs=4, space="PSUM") as cxp,
                tc.tile_pool(name="wk", bufs=6) as work,
            ):
                for qi in range(16):
                    qc, st = qi // 4, qi % 4
                    cs = cxp.tile([128, 2, 512], F32, tag="cs")
                    for jp in range(JP):
                        lhs = pt[jp][:, qc, :, st * 128 : (st + 1) * 128]
                        nc.tensor.matmul(
                            cs[:, 0, 0:256],
                            lhs,
                            v8[:, jp, :, 0:256],
                            start=(jp == 0),
                            stop=(jp == JP - 1),
                            perf_mode=DR,
                        )
                        nc.tensor.matmul(
                            cs[:, 1, 0:257],
                            lhs,
                            v8[:, jp, :, 256 : E + 1],
                            start=(jp == 0),
                            stop=(jp == JP - 1),
                            perf_mode=DR,
                        )
                    recip = work.tile([128, 1], F32, tag="recip")
                    nc.vector.reciprocal(recip, cs[:, 1, 256:257])
                    h = work.tile([128, E], F32, tag="h")
                    nc.vector.scalar_tensor_tensor(
                        out=h[:, 0:256],
                        in0=cs[:, 0, 0:256],
                        scalar=recip,
                        in1=xqst[:, qi, 0:256],
                        op0=OP.mult,
                        op1=OP.add,
                    )
                    tb = work.tile([128, 256], F32, tag="tb")
                    nc.scalar.activation(
                        out=tb, in_=cs[:, 1, 0:256], func=AF.Identity, scale=recip
                    )
                    nc.gpsimd.tensor_add(h[:, 256:512], tb, xqst[:, qi, 256:512])
                    st6 = work.tile([128, 6], F32, tag="st6")
                    nc.vector.bn_stats(out=st6, in_=h)
                    mv = work.tile([128, 2], F32, tag="mv")
                    nc.vector.bn_aggr(out=mv, in_=st6)
                    std = work.tile([128, 1], F32, tag="std")
                    nc.scalar.activation(
                        out=std, in_=mv[:, 1:2], func=AF.Sqrt, bias=eps_t
                    )
                    rstd = work.tile([128, 1], F32, tag="rstd")
                    nc.vector.reciprocal(rstd, std)
                    nmu = work.tile([128, 1], F32, tag="nmu")
                    nc.gpsimd.tensor_scalar(
                        out=nmu,
                        in0=mv[:, 0:1],
                        scalar1=rstd,
                        scalar2=-1.0,
                        op0=OP.mult,
                        op1=OP.mult,
                    )
                    o_t = work.tile([128, E], BF16, tag="ot")
                    nc.scalar.activation(
                        out=o_t[:, 0:256],
                        in_=h[:, 0:256],
                        func=AF.Identity,
                        bias=nmu,
                        scale=rstd,
                    )
                    nc.gpsimd.tensor_scalar(
                        out=o_t[:, 256:512],
                        in0=h[:, 256:512],
                        scalar1=rstd,
                        scalar2=nmu,
                        op0=OP.mult,
                        op1=OP.add,
                    )
                    if apply_gb:
                        nc.vector.tensor_mul(o_t, o_t, ga_bc)
                        nc.vector.tensor_add(o_t, o_t, be_bc)
                    nc.sync.dma_start(
                        out=out[qi * 128 : (qi + 1) * 128, :], in_=o_t
                    )
    return nc


# test-harness knobs (the grading harness leaves these at defaults)
TRACE = False
LAST_RESULTS = None


def _ensure_axon_jax():
    """The Bass SPMD run goes through jax/PJRT on the axon platform. If the
    caller pinned jax to cpu (e.g. to run a reference model), unpin it and
    drop any initialized cpu-only backends."""
    import os

    import jax

    try:
        devs = jax.devices()
    except Exception:
        devs = []
    if any(d.platform not in ("cpu",) for d in devs):
        return
    os.environ.pop("JAX_PLATFORMS", None)
    try:
        jax.config.update("jax_platforms", None)
    except Exception:
        pass
    try:
        jax.clear_backends()
    except Exception:
        try:
            jax.extend.backend.clear_backends()
        except Exception:
            pass


def _img_w(w):
    # [E, E] (row = contract e) -> SBUF image [128, 2(mc), 2(i), 512] flat
    return np.ascontiguousarray(
        w.reshape(2, 2, 128, E).transpose(2, 0, 1, 3).reshape(128, -1)
    )


def _img_x8(xT, nchunk):
    # [E, N] fp8 (row = contract e) -> image [128, 2(mc), nchunk, 2(i), 512]
    return np.ascontiguousarray(
        xT.reshape(2, 2, 128, nchunk, 512)
        .transpose(2, 0, 3, 1, 4)
        .reshape(128, -1)
    )


def kernel(x, mask, Wq, bq, Wk, bk, Wv, bv, gamma, beta):
    global LAST_RESULTS
    _ensure_axon_jax()
    from concourse.bass_utils import run_bass_kernel_spmd

    x = np.ascontiguousarray(np.asarray(x, dtype=np.float32))
    mask = np.asarray(np.asarray(mask) != 0)
    Wq = np.asarray(Wq, dtype=np.float32)
    Wk = np.asarray(Wk, dtype=np.float32)
    Wv = np.asarray(Wv, dtype=np.float32)
    bq = np.asarray(bq, dtype=np.float32)
    # Masked keys get softmax weight exactly 0 (exp underflow), so attention
    # only needs the unmasked keys: pack them per batch, padded to a 128
    # multiple; pad slots get the -1e4 bias -> exp==0 (odd tile counts are
    # rounded up to even for the DoubleRow pairing, pad tile P memset to 0).
    counts = [int(mask[b].sum()) for b in range(4)]
    nkt_real = max(2, -(-max(counts) // 128))
    nkt2 = nkt_real + (nkt_real % 2)
    KC = (nkt2 + 3) // 4
    SKC = KC * 512  # xkv image column span (512-aligned)
    bf16 = ml_dtypes.bfloat16
    fp8 = ml_dtypes.float8_e4m3
    # fused QK weights: scores^T = x_k (G x_q^T), G = Wq^T Wk (x64 for fp8
    # range; divided back out in the exp scale). bq enters scores only via
    # the per-key bias x_k.(Wk^T bq), folded into maskbias in exact fp32.
    G8 = _img_w((G_SCALE * (Wq.T @ Wk)).astype(fp8))
    Wv8 = _img_w((V_SCALE * Wv.T).astype(fp8))
    c_bias = Wk.T @ bq  # [E]
    common = {
        "G8": G8,
        "Wv8": Wv8,
        "gamma": np.ascontiguousarray(gamma, dtype=np.float32),
        "beta": np.ascontiguousarray(beta, dtype=np.float32),
    }

    # residual carries x + bv (exact: ctx/rs + bv + x == (ctx incl. bv)/rs + x)
    xres16 = (x + np.asarray(bv, dtype=np.float32)).astype(bf16)
    x8 = x.astype(fp8)
    in_maps = []
    for b in range(4):
        sel = mask[b]
        xsel = x[b][sel]
        sel8 = x8[b][sel]
        xkv8h = np.zeros((SKC, E), dtype=fp8)
        xkv8h[: len(sel8)] = sel8
        xkv8img = _img_x8(np.ascontiguousarray(xkv8h.T), KC)
        mb = np.full(nkt2 * 128, MASK_NEG + SHIFT, dtype=np.float32)
        mb[: len(sel8)] = SHIFT + SCALE * (xsel @ c_bias)
        mbimg = np.ascontiguousarray(mb.reshape(nkt2, 128).T)
        for hh in range(2):
            xh8 = x8[b, hh * SQ : (hh + 1) * SQ]  # [SQ, E]
            xq8img = _img_x8(np.ascontiguousarray(xh8.T), QC)
            xrimg = np.ascontiguousarray(
                xres16[b, hh * SQ : (hh + 1) * SQ]
                .reshape(16, 128, E)
                .transpose(1, 0, 2)
                .reshape(128, -1)
            )
            in_maps.append(
                {
                    "xr": xrimg,
                    "xq8": xq8img,
                    "xkv8": xkv8img,
                    "mb": mbimg,
                    **common,
                }
            )
    apply_gb = not (
        np.all(np.asarray(gamma) == 1.0) and np.all(np.asarray(beta) == 0.0)
    )
    nc = build_nc(nkt_real, nkt2, apply_gb)
    nc.compile()
    res = run_bass_kernel_spmd(nc, in_maps, core_ids=list(range(8)), trace=TRACE)
    LAST_RESULTS = res
    full = np.empty((4, S, E), dtype=np.float32)
    for c in range(8):
        b, hh = c // 2, c % 2
        full[b, hh * SQ : (hh + 1) * SQ] = res.results[c]["out"].astype(np.float32)
    return full


# revision 15
# speedup vs baseline: 1.1957x; 1.1957x over previous
"""Fused single-head attention + residual + LayerNorm for Trainium2 (Bass/Tile).

Problem: B=4, S=4096, E=512 fp32.
  Q/K/V = x @ W^T + b ; S = QK^T/sqrt(E) ; mask keys ; softmax ; ctx = P@V ;
  out = LayerNorm(ctx + x) * gamma + beta

Sharding: 8 cores = 4 batches x 2 halves of the S=4096 query rows
(sequence-parallel attention). Masked keys carry exactly zero softmax
weight, so each core receives only the PACKED (unmasked) key rows of its
whole batch.

Kernel strategy (v4):
  - The Q and K projections are FUSED on the host: scores^T = x_k G x_q^T
    with G = Wq^T Wk precomputed in fp32 and quantized once (x64 scaling
    keeps G out of the fp8 subnormal range; the 1/64 rides the exp scale).
    The device computes T = G x_q^T (same cost as the old Q projection)
    and contracts it against the resident x_k fp8 tiles -- the whole K
    projection disappears. bq enters scores only as a per-key bias
    bq.K[k] = x_k.(Wk^T bq), folded into the host-built mask-bias vector
    in exact fp32; bk is per-query (softmax-invariant, dropped); bv is
    pre-added to the bf16 residual. Wv is scaled x32 (fp8 subnormal
    range); the P-rowsum ones-column is 32 so one reciprocal cancels all.
  - Every input arrives as a per-tile SBUF image (partition-major,
    contiguous) so each load is a dense 128-descriptor DMA; operand tiles
    are chunked to the order the PE consumes them, issues are spread over
    the sync/scalar/gpsimd queues, and the 2MB bf16 residual is deferred.
    Gap-free feeding also keeps the PE out of its cold p-state (a stall
    resets a ~3us ramp to full clock).
  - Middle phase: transposed score tiles [128k, 2qc, 512q] cycle through a
    3-deep 6-bank PSUM ring, paced by the ScalarE exp wall (~1us per
    half-tile, table pre-warmed at t=0); the V projection runs in its own
    1-deep 2-bank ring, emitted after each 2-k-tile group's score matmuls
    so it fills PE slack without delaying exp. P is written straight into
    the paired fp8 ctx layout.
  - ctx accumulates over all k-tiles per 128-query chunk; rowsum rides as
    V column 512. LayerNorm is spread over four engines so no queue runs
    above the PE's 1.9us/chunk pace: DVE recip/h-half/split bn_stats,
    ScalarE scale-half + sqrt + normalize-half, GpSimd residual-add +
    -mu*rstd + normalize-half, output DMA issued from the idle sync queue.
    bn_stats runs per half so stats begin before the residual-add of the
    second half lands.
"""

import sys

import ml_dtypes
import numpy as np

sys.path.insert(0, "/opt/trn_rl_repo")

import concourse.bass as bass  # noqa: E402
import concourse.tile as tile  # noqa: E402
from concourse import bacc, mybir  # noqa: E402

E = 512
S = 4096  # keys per batch
SQ = 2048  # query rows per core
QC = SQ // 512  # 4   512-chunks along q
F32 = mybir.dt.float32
BF16 = mybir.dt.bfloat16
FP8 = mybir.dt.float8e4
SCALE = 1.0 / float(np.sqrt(E))
G_SCALE = 64.0  # G quantized as 64*(Wq^T Wk); exp scale divides it out
V_SCALE = 32.0  # Wv quantized as 32*Wv^T; ones-col=32 cancels via recip
EPS = 1e-5
MASK_NEG = -10000.0
SHIFT = -1.0  # softmax-invariant score shift, keeps exp() in fp8 range
DR = mybir.MatmulPerfMode.DoubleRow


def build_nc(nkt_real, nkt2, apply_gb):
    # nkt_real = k-tiles of 128 packed keys; nkt2 = even-rounded (DR pairs)
    assert nkt2 % 2 == 0 and nkt_real in (nkt2, nkt2 - 1)
    SK = nkt2 * 128
    JP = nkt2 // 2  # ctx pair-tiles of 256 keys
    KC = (nkt2 + 3) // 4  # xkv 512-column chunks

    nc = bacc.Bacc("TRN2", target_bir_lowering=False, debug=False)
    # inputs are per-tile SBUF images: [128 partitions, tile-free...] dense
    xr = nc.dram_tensor("xr", [128, 16 * E], BF16, kind="ExternalInput")
    xq8d = nc.dram_tensor("xq8", [128, 2 * QC * 2 * 512], FP8, kind="ExternalInput")
    xkv8d = nc.dram_tensor(
        "xkv8", [128, 2 * KC * 2 * 512], FP8, kind="ExternalInput"
    )
    mbd = nc.dram_tensor("mb", [128, nkt2], F32, kind="ExternalInput")
    G8d = nc.dram_tensor("G8", [128, 2048], FP8, kind="ExternalInput")
    Wv8d = nc.dram_tensor("Wv8", [128, 2048], FP8, kind="ExternalInput")
    gamma = nc.dram_tensor("gamma", [E], F32, kind="ExternalInput")
    beta = nc.dram_tensor("beta", [E], F32, kind="ExternalInput")
    out = nc.dram_tensor("out", [SQ, E], BF16, kind="ExternalOutput")

    AF = mybir.ActivationFunctionType
    OP = mybir.AluOpType

    with tile.TileContext(nc) as tc:
        with tc.tile_pool(name="persist", bufs=1) as persist:
            # ---------------- constants / persistent tiles ----------------
            mbcols = persist.tile([128, nkt2], F32, tag="mb")
            eps_t = persist.tile([128, 1], F32, tag="eps")
            nc.vector.memset(eps_t, EPS)
            dummy = persist.tile([128, 1], F32, tag="dummy")
            if apply_gb:
                ga_bc = persist.tile([128, E], F32, tag="gabc")
                be_bc = persist.tile([128, E], F32, tag="bebc")
                nc.gpsimd.dma_start(
                    out=ga_bc, in_=bass.AP(tensor=gamma[:].tensor, offset=0, ap=[[0, 128], [1, E]])
                )
                nc.gpsimd.dma_start(
                    out=be_bc, in_=bass.AP(tensor=beta[:].tensor, offset=0, ap=[[0, 128], [1, E]])
                )

            # fp8 operand tiles (paired [.., 2, ..] layouts)
            # g8/wv8: [128e, 2(mc), 2(i), 512f]; contract e = 256mc + 128i + p
            g8 = persist.tile([128, 2, 2, E], FP8, name="g8", tag="g8")
            wv8 = persist.tile([128, 2, 2, E], FP8, name="wv8", tag="wv8")
            # xq8[mc][qc]: [128e, 2(i), 512q]
            xq8 = [
                [
                    persist.tile(
                        [128, 2, 512], FP8, name=f"xq8_{m}_{c}", tag=f"xq8_{m}_{c}"
                    )
                    for c in range(QC)
                ]
                for m in range(2)
            ]
            # xkv8[mc][c]: [128e, 2(i), 512k]  (k-chunk of 4 k-tiles)
            xkv8 = [
                [
                    persist.tile(
                        [128, 2, 512], FP8, name=f"xkv8_{m}_{c}", tag=f"xkv8_{m}_{c}"
                    )
                    for c in range(KC)
                ]
                for m in range(2)
            ]
            # T8[mo]: [128f, 2(i), SQ]  (T = 64*G x_q^T; f = 256mo + 128i + p)
            T8 = [
                persist.tile([128, 2, SQ], FP8, name=f"T8_{m}", tag=f"T8_{m}")
                for m in range(2)
            ]
            # v8: [128k, JP, 2(i), 513]  (k = 256jp + 128i + p; col 512 = 32)
            v8 = persist.tile([128, JP, 2, E + 1], FP8, name="v8", tag="v8")
            # P tiles: pt[jp]: [128k, 4(qc), 2(i), 512q]
            pt = [
                persist.tile([128, 4, 2, 512], FP8, name=f"pt{j}", tag=f"pt{j}")
                for j in range(JP)
            ]
            # residual x staged for the LN phase: [128, 16(qi), 512]
            xqst = persist.tile([128, 16, E], BF16, name="xqst", tag="xqst")

            # ---- critical loads first: weights lead the scalar queue,
            # xq8 per-tile chunks on sync, xkv8 on gpsimd, residual deferred
            nc.sync.dma_start(out=g8, in_=G8d[:, :])
            nc.scalar.dma_start(out=wv8, in_=Wv8d[:, :])
            for qc in range(QC):
                for m in range(2):
                    nc.sync.dma_start(
                        out=xq8[m][qc],
                        in_=xq8d[:, (m * QC + qc) * 1024 : (m * QC + qc + 1) * 1024],
                    )
            for c in range(KC):
                for m in range(2):
                    nc.gpsimd.dma_start(
                        out=xkv8[m][c],
                        in_=xkv8d[:, (m * KC + c) * 1024 : (m * KC + c + 1) * 1024],
                    )
            nc.gpsimd.dma_start(out=mbcols, in_=mbd[:, :])

            # pre-warm the exp activation table while DMAs stream
            nc.scalar.activation(out=dummy, in_=eps_t, func=AF.Exp)

            # rowsum ones-columns (=32 to cancel the Wv x32) on idle GpSimd
            nc.gpsimd.memset(v8[:, :, :, E : E + 1], V_SCALE)
            if nkt_real < nkt2:
                # pad k-tile never exp'd: zero its P so ctx sees 0 weight
                nc.gpsimd.memset(pt[JP - 1][:, :, 1, :], 0.0)
            # deferred bulk: bf16 residual (first needed by the LN phase)
            nc.gpsimd.dma_start(out=xqst, in_=xr[:, :])

            def kslice(kt):
                c, r = kt // 4, kt % 4
                return [xkv8[m][c][:, :, r * 128 : (r + 1) * 128] for m in range(2)]

            with (
                tc.tile_pool(name="sp", bufs=3, space="PSUM") as spp,
                tc.tile_pool(name="vp", bufs=1, space="PSUM") as vpp,
            ):
                def t_tile(qc, mo):
                    ps = spp.tile([128, 2, 512], F32, tag="sc")
                    for i in range(2):
                        ft = 2 * mo + i
                        for mc in range(2):
                            nc.tensor.matmul(
                                ps[:, i, :],
                                g8[:, mc, :, ft * 128 : (ft + 1) * 128],
                                xq8[mc][qc],
                                start=(mc == 0),
                                stop=(mc == 1),
                                perf_mode=DR,
                            )
                    # cast the two halves on ScalarE and DVE in parallel --
                    # halved latency keeps the 3-deep ring ahead of the MMs
                    nc.scalar.copy(
                        out=T8[mo][:, 0, qc * 512 : (qc + 1) * 512], in_=ps[:, 0, :]
                    )
                    nc.vector.tensor_copy(
                        T8[mo][:, 1, qc * 512 : (qc + 1) * 512], ps[:, 1, :]
                    )

                def score_half(kt, h):
                    ks = kslice(kt)
                    sc = spp.tile([128, 2, 512], F32, tag="sc")
                    for mc in range(2):
                        for qq in range(2):
                            qc = 2 * h + qq
                            nc.tensor.matmul(
                                sc[:, qq, :],
                                ks[mc],
                                T8[mc][:, :, qc * 512 : (qc + 1) * 512],
                                start=(mc == 0),
                                stop=(mc == 1),
                                perf_mode=DR,
                            )
                    nc.scalar.activation(
                        out=pt[kt // 2][:, 2 * h : 2 * h + 2, kt % 2, :],
                        in_=sc,
                        func=AF.Exp,
                        bias=mbcols[:, kt : kt + 1],
                        scale=SCALE / G_SCALE,
                    )

                def v_group(g):
                    # V projection for k-tiles 2g, 2g+1 (PE stall-gap filler,
                    # own 1-deep ring so it never blocks a score tile)
                    ps = vpp.tile([128, 2, 512], F32, tag="vt")
                    for i in range(2):
                        if 2 * g + i >= nkt2:
                            continue
                        ks = kslice(2 * g + i)
                        for mc in range(2):
                            nc.tensor.matmul(
                                ps[:, i, :],
                                ks[mc],
                                wv8[:, mc, :, :],
                                start=(mc == 0),
                                stop=(mc == 1),
                                perf_mode=DR,
                            )
                    nc.vector.tensor_copy(v8[:, g, :, 0:E], ps)

                # ---- T = G x_q^T (fused QK projection) ----
                for qc in range(QC):
                    for mo in range(2):
                        t_tile(qc, mo)

                # ---- scores + exp (ScalarE wall) with V-proj interleaved ----
                for g in range(JP):
                    for i in range(2):
                        kt = 2 * g + i
                        if kt >= nkt_real:
                            continue
                        for h in range(2):
                            score_half(kt, h)
                    v_group(g)

            # ---------------- ctx + residual + layernorm ----------------
            with (
                tc.tile_pool(name="cx", bufs=4, space="PSUM") as cxp,
                tc.tile_pool(name="wk", bufs=6) as work,
            ):
                for qi in range(16):
                    qc, st = qi // 4, qi % 4
                    cs = cxp.tile([128, 2, 512], F32, tag="cs")
                    for jp in range(JP):
                        lhs = pt[jp][:, qc, :, st * 128 : (st + 1) * 128]
                        nc.tensor.matmul(
                            cs[:, 0, 0:256],
                            lhs,
                            v8[:, jp, :, 0:256],
                            start=(jp == 0),
                            stop=(jp == JP - 1),
                            perf_mode=DR,
                        )
                        nc.tensor.matmul(
                            cs[:, 1, 0:257],
                            lhs,
                            v8[:, jp, :, 256 : E + 1],
                            start=(jp == 0),
                            stop=(jp == JP - 1),
                            perf_mode=DR,
                        )
                    recip = work.tile([128, 1], F32, tag="recip")
                    nc.vector.reciprocal(recip, cs[:, 1, 256:257])
                    h = work.tile([128, E], F32, tag="h")
                    nc.vector.scalar_tensor_tensor(
                        out=h[:, 0:256],
                        in0=cs[:, 0, 0:256],
                        scalar=recip,
                        in1=xqst[:, qi, 0:256],
                        op0=OP.mult,
                        op1=OP.add,
                    )
                    tb = work.tile([128, 256], F32, tag="tb")
                    nc.scalar.activation(
                        out=tb, in_=cs[:, 1, 0:256], func=AF.Identity, scale=recip
                    )
                    nc.gpsimd.tensor_add(h[:, 256:512], tb, xqst[:, qi, 256:512])
                    st6 = work.tile([128, 6], F32, tag="st6")
                    nc.vector.bn_stats(out=st6, in_=h)
                    mv = work.tile([128, 2], F32, tag="mv")
                    nc.vector.bn_aggr(out=mv, in_=st6)
                    std = work.tile([128, 1], F32, tag="std")
                    nc.scalar.activation(
                        out=std, in_=mv[:, 1:2], func=AF.Sqrt, bias=eps_t
                    )
                    rstd = work.tile([128, 1], F32, tag="rstd")
                    nc.vector.reciprocal(rstd, std)
                    nmu = work.tile([128, 1], F32, tag="nmu")
                    nc.gpsimd.tensor_scalar(
                        out=nmu,
                        in0=mv[:, 0:1],
                        scalar1=rstd,
                        scalar2=-1.0,
                        op0=OP.mult,
                        op1=OP.mult,
                    )
                    o_t = work.tile([128, E], BF16, tag="ot")
                    nc.scalar.activation(
                        out=o_t[:, 0:256],
                        in_=h[:, 0:256],
                        func=AF.Identity,
                        bias=nmu,
                        scale=rstd,
                    )
                    nc.gpsimd.tensor_scalar(
                        out=o_t[:, 256:512],
                        in0=h[:, 256:512],
                        scalar1=rstd,
                        scalar2=nmu,
                        op0=OP.mult,
                        op1=OP.add,
                    )
                    if apply_gb:
                        nc.vector.tensor_mul(o_t, o_t, ga_bc)
                        nc.vector.tensor_add(o_t, o_t, be_bc)
                    nc.sync.dma_start(
                        out=out[qi * 128 : (qi + 1) * 128, :], in_=o_t
                    )
    return nc


# test-harness knobs (the grading harness leaves these at defaults)
TRACE = False
LAST_RESULTS = None


def _ensure_axon_jax():
    """The Bass SPMD run goes through jax/PJRT on the axon platform. If the
    caller pinned jax to cpu (e.g. to run a reference model), unpin it and
    drop any initialized cpu-only backends."""
    import os

    import jax

    try:
        devs = jax.devices()
    except Exception:
        devs = []
    if any(d.platform not in ("cpu",) for d in devs):
        return
    os.environ.pop("JAX_PLATFORMS", None)
    try:
        jax.config.update("jax_platforms", None)
    except Exception:
        pass
    try:
        jax.clear_backends()
    except Exception:
        try:
            jax.extend.backend.clear_backends()
        except Exception:
            pass


def _img_w(w):
    # [E, E] (row = contract e) -> SBUF image [128, 2(mc), 2(i), 512] flat
    return np.ascontiguousarray(
        w.reshape(2, 2, 128, E).transpose(2, 0, 1, 3).reshape(128, -1)
    )


def _img_x8(xT, nchunk):
    # [E, N] fp8 (row = contract e) -> image [128, 2(mc), nchunk, 2(i), 512]
    return np.ascontiguousarray(
        xT.reshape(2, 2, 128, nchunk, 512)
        .transpose(2, 0, 3, 1, 4)
        .reshape(128, -1)
    )


def kernel(x, mask, Wq, bq, Wk, bk, Wv, bv, gamma, beta):
    global LAST_RESULTS
    _ensure_axon_jax()
    from concourse.bass_utils import run_bass_kernel_spmd

    x = np.ascontiguousarray(np.asarray(x, dtype=np.float32))
    mask = np.asarray(np.asarray(mask) != 0)
    Wq = np.asarray(Wq, dtype=np.float32)
    Wk = np.asarray(Wk, dtype=np.float32)
    Wv = np.asarray(Wv, dtype=np.float32)
    bq = np.asarray(bq, dtype=np.float32)
    # Masked keys get softmax weight exactly 0 (exp underflow), so attention
    # only needs the unmasked keys: pack them per batch, padded to a 128
    # multiple; pad slots get the -1e4 bias -> exp==0 (odd tile counts are
    # rounded up to even for the DoubleRow pairing, pad tile P memset to 0).
    counts = [int(mask[b].sum()) for b in range(4)]
    nkt_real = max(2, -(-max(counts) // 128))
    nkt2 = nkt_real + (nkt_real % 2)
    KC = (nkt2 + 3) // 4
    SKC = KC * 512  # xkv image column span (512-aligned)
    bf16 = ml_dtypes.bfloat16
    fp8 = ml_dtypes.float8_e4m3
    # fused QK weights: scores^T = x_k (G x_q^T), G = Wq^T Wk (x64 for fp8
    # range; divided back out in the exp scale). bq enters scores only via
    # the per-key bias x_k.(Wk^T bq), folded into maskbias in exact fp32.
    G8 = _img_w((G_SCALE * (Wq.T @ Wk)).astype(fp8))
    Wv8 = _img_w((V_SCALE * Wv.T).astype(fp8))
    c_bias = Wk.T @ bq  # [E]
    common = {
        "G8": G8,
        "Wv8": Wv8,
        "gamma": np.ascontiguousarray(gamma, dtype=np.float32),
        "beta": np.ascontiguousarray(beta, dtype=np.float32),
    }

    # residual carries x + bv (exact: ctx/rs + bv + x == (ctx incl. bv)/rs + x)
    xres16 = (x + np.asarray(bv, dtype=np.float32)).astype(bf16)
    x8 = x.astype(fp8)
    in_maps = []
    for b in range(4):
        sel = mask[b]
        xsel = x[b][sel]
        sel8 = x8[b][sel]
        xkv8h = np.zeros((SKC, E), dtype=fp8)
        xkv8h[: len(sel8)] = sel8
        xkv8img = _img_x8(np.ascontiguousarray(xkv8h.T), KC)
        mb = np.full(nkt2 * 128, MASK_NEG + SHIFT, dtype=np.float32)
        mb[: len(sel8)] = SHIFT + SCALE * (xsel @ c_bias)
        mbimg = np.ascontiguousarray(mb.reshape(nkt2, 128).T)
        for hh in range(2):
            xh8 = x8[b, hh * SQ : (hh + 1) * SQ]  # [SQ, E]
            xq8img = _img_x8(np.ascontiguousarray(xh8.T), QC)
            xrimg = np.ascontiguousarray(
                xres16[b, hh * SQ : (hh + 1) * SQ]
                .reshape(16, 128, E)
                .transpose(1, 0, 2)
                .reshape(128, -1)
            )
            in_maps.append(
                {
                    "xr": xrimg,
                    "xq8": xq8img,
                    "xkv8": xkv8img,
                    "mb": mbimg,
                    **common,
                }
            )
    apply_gb = not (
        np.all(np.asarray(gamma) == 1.0) and np.all(np.asarray(beta) == 0.0)
    )
    nc = build_nc(nkt_real, nkt2, apply_gb)
    nc.compile()
    res = run_bass_kernel_spmd(nc, in_maps, core_ids=list(range(8)), trace=TRACE)
    LAST_RESULTS = res
    full = np.empty((4, S, E), dtype=np.float32)
    for c in range(8):
        b, hh = c // 2, c % 2
        full[b, hh * SQ : (hh + 1) * SQ] = res.results[c]["out"].astype(np.float32)
    return full
